# revision 44
# baseline (speedup 1.0000x reference)
"""Trainium2 Bass kernel for nn_DelayExpansionLayer (histogram_binning).

Computation: per-channel mean of layer_output [64,256,56,56] over (B,H,W),
round to 1e-6, nearest-key lookup in a sorted 1024-entry table, max over
channels, scale by (in_ch*out_ch)/512, broadcast to (56,56).

Strategy (data-parallel over batch, 8 NeuronCores):
  - Each core gets 8 batches = [8,256,56,56] (25.7 MB) and computes
    per-channel partial sums [256] on-device (DMA-bound reduction).
  - Host combines the 8 partial-sum vectors (the tiny [C] all-reduce),
    then does the O(C+K) lookup/max/broadcast epilogue.

Per-core device kernel (raw bass, manual semaphores):
  input  x [8, 128, 2, 3136] f32  (batch, partition, chan-pair, spatial).
  The HW DGE sprays a DMA's partition dim over the 16 SDMA engines;
  engine 15 (E79) also runs the dynamic queues' bookkeeping and moves
  bytes ~20% slower (1.606MB/engine -> ~74us on E79 vs ~61us on the
  rest), and a queue executes its DMA instructions ~serially, adding a
  per-instruction gap while completions collect.  So: keep every
  instruction full 128-wide (25KB/partition packets; narrower splits
  serialize badly), and alternate whole batches between the TWO hardware
  queues -- Q-sync: b0, b2, b4, b6 + tail chunks c0/c2; Q-scalar: b1,
  b3, b5 + tail chunks c1/c3 -- so one queue's packets fill the other's
  instruction-boundary gaps and E79 streams back-to-back.  Six fresh
  slots avoid all slot-reuse stalls except b6 (waits b0's reduce).
  Batch 7 is tapered (j0 full, then j1 as 1568/784/784) so the last
  reduce lands right after the last byte.  Reduction is split across
  DVE (tensor_reduce) and ACT (activation-Copy accum), with the late
  work j-split so both engines finish ~3us after the stream: DVE takes
  b0,b2,b4 pairs + b6-j0 + c0[0:1568] + c1; ACT takes b1,b3,b5 pairs +
  b6-j1 + c0[1568:] + c2,c3.  Partial sums stats[128,2,10] go out in
  two DMAs on Q-sync (early cols 0-5, final cols 6-9; the scalar
  queue's completion path is ~2us slower for tiny transfers); channel
  c = 2*p + j.  Quiet-window budget: ~9us framework entry + ~61us
  stream (16 engines x 26.5 GB/s per-core cap) + ~7us reduce/out tail.
"""

import sys
import types

import numpy as np

N_CORES = 8
B_FULL, C, H, W = 64, 256, 56, 56
HW = H * W
B_LOCAL = B_FULL // N_CORES
SCALE_DENOM = 32 * 16

# Set by a test harness to enable NTFF tracing of the SPMD run.
TRACE = False
TRACE_TMPDIR = None
LAST_RESULTS = None

_CACHE = {}


def _ensure_axon_hooks_shim():
    """bass_utils' axon trace path imports antenv.axon_hooks; provide a
    no-op shim when the environment's antenv package lacks it."""
    try:
        import antenv.axon_hooks  # noqa: F401
        return
    except ImportError:
        pass

    mod = types.ModuleType("antenv.axon_hooks")
    _hook = [None]
    mod.set_axon_ntff_profile_hook = lambda h: _hook.__setitem__(0, h)
    mod.get_axon_ntff_profile_hook = lambda: _hook[0]
    sys.modules["antenv.axon_hooks"] = mod
    try:
        import antenv

        antenv.axon_hooks = mod
    except ImportError:
        pass


def _build():
    if "nc" in _CACHE:
        return _CACHE["nc"]
    import concourse.bass as bass
    from concourse import mybir

    nc = bass.Bass(
        "TRN2",
        target_bir_lowering=False,
        debug=False,
        enable_asserts=False,
        num_devices=N_CORES,
        # trim the entry preamble: no partition-id parameter load (the
        # kernel has no per-core branches) and no monotonic semaphores
        enable_partition_id=False,
        monotonic_sem_count=0,
    )
    f32 = mybir.dt.float32
    x = nc.dram_tensor("x", [B_LOCAL, 128, 2, HW], f32, kind="ExternalInput").ap()
    out = nc.dram_tensor("out", [128, 2, 10], f32, kind="ExternalOutput").ap()

    # 6 full batch slots (b0-b5; b6 reuses s0 after b0's reduce) + exact-
    # size tail buffers for b7's tapered chunks + stats.
    slots = [
        nc.alloc_sbuf_tensor(f"slot{i}", [128, 2, HW], f32).ap() for i in range(7)
    ]
    t0 = nc.alloc_sbuf_tensor("t0", [128, HW], f32).ap()
    t1 = nc.alloc_sbuf_tensor("t1", [128, 1568], f32).ap()
    t2 = nc.alloc_sbuf_tensor("t2", [128, 784], f32).ap()
    t3 = nc.alloc_sbuf_tensor("t3", [128, 784], f32).ap()
    stats = nc.alloc_sbuf_tensor("stats", [128, 2, 10], f32).ap()

    # b7 taper chunks: (j, s0, s1, dest tile, stats col)
    TAIL = (
        (0, 0, HW, t0, 7),
        (1, 0, 1568, t1, 7),
        (1, 1568, 2352, t2, 8),
        (1, 2352, HW, t3, 9),
    )

    with (
        nc.Block(no_gpsimd_drain=True) as block,
        nc.semaphore("ds0") as ds0,
        nc.semaphore("ds1") as ds1,
        nc.semaphore("ds2") as ds2,
        nc.semaphore("ds3") as ds3,
        nc.semaphore("ds4") as ds4,
        nc.semaphore("ds5") as ds5,
        nc.semaphore("ds6") as ds6,
        nc.semaphore("dt0") as dt0,
        nc.semaphore("dt1") as dt1,
        nc.semaphore("dt2") as dt2,
        nc.semaphore("dt3") as dt3,
        nc.semaphore("vd") as vd,
        nc.semaphore("ad") as ad,
        nc.semaphore("od") as od,
    ):
        dt = [dt0, dt1, dt2, dt3]

        @block.sync
        def _(sync: bass.BassEngine):
            # Q-sync: b0, b6, b2, b4 into fresh slots, no deps.  b6
            # streams EARLY (fresh slot s6, no reuse gate) so its ~6us
            # of j-split reduces run fully hidden mid-stream; only the
            # small taper chunks trail the last byte.
            for b, slot, sem in (
                (0, slots[0], ds0),
                (6, slots[6], ds6),
            ):
                sync.dma_start(out=slot[:], in_=x[b]).then_inc(sem, 16)
            # tail chunk c0 (j0 full) 3rd in the queue: its 2us of
            # reduce work runs hidden while b2/b4 stream
            j0_, s0_, s1_, tile_, _k = TAIL[0]
            sync.dma_start(
                out=tile_[:, 0 : s1_ - s0_], in_=x[7, :, j0_, s0_:s1_]
            ).then_inc(dt0, 16)
            for b, slot, sem in (
                (2, slots[2], ds2),
                (4, slots[4], ds4),
            ):
                sync.dma_start(out=slot[:], in_=x[b]).then_inc(sem, 16)
            # tail chunk c2 (j1 1568:2352) last
            j, s0, s1, tile, _k = TAIL[2]
            sync.dma_start(
                out=tile[:, 0 : s1 - s0], in_=x[7, :, j, s0:s1]
            ).then_inc(dt2, 16)
            # early out-DMA for batch columns 0..5 once their reduces
            # done (vd>=5 covers b0,b6,c0a,b2,b4-j0; ad>=6 covers
            # b1,b6,c0b,b3,b5,b4-j1)
            sync.wait_ge(vd, 5)
            sync.wait_ge(ad, 6)
            sync.dma_start(
                out=out[:, :, 0:6], in_=stats[:, :, 0:6], single_packet=True
            ).then_inc(
                od, 16
            )
            # final out-DMA (cols 6..9).  ad>=8 orders it after the last
            # ACTIVATE's accumulator writeback (the update fires post-
            # writeback); vd>=6 after DVE's last reduce.
            sync.wait_ge(ad, 8)
            sync.wait_ge(vd, 6)
            sync.dma_start(
                out=out[:, :, 6:10], in_=stats[:, :, 6:10], single_packet=True
            ).then_inc(
                od, 16
            )
            sync.wait_ge(od, 32)

        @block.vector
        def _(vector: bass.BassEngine):
            # b0, then b6-j0 (b6 streams 2nd; ACT takes its j1), then
            # b2, b4 pair reduces -- all hidden mid-stream
            vector.wait_ge(ds0, 16)
            vector.reduce_sum(
                stats[:, :, 0:1], slots[0][:], axis=mybir.AxisListType.X
            ).then_inc(vd, 1)
            vector.wait_ge(ds6, 16)
            vector.reduce_sum(
                stats[:, 0, 6:7], slots[6][:, 0, :], axis=mybir.AxisListType.X
            ).then_inc(vd, 1)
            # tail c0 first half (j0 cols 0:1568 -> j0 col 7), hidden
            # while b2/b4 stream
            vector.wait_ge(dt0, 16)
            vector.reduce_sum(
                stats[:, 0, 7:8], t0[:, 0:1568], axis=mybir.AxisListType.X
            ).then_inc(vd, 1)
            vector.wait_ge(ds2, 16)
            vector.reduce_sum(
                stats[:, :, 2:3], slots[2][:], axis=mybir.AxisListType.X
            ).then_inc(vd, 1)
            # b4 is Q-sync's last full batch (lands near stream end on
            # loaded runs) -- j-split its reduce: DVE j0, ACT j1
            vector.wait_ge(ds4, 16)
            vector.reduce_sum(
                stats[:, 0, 4:5], slots[4][:, 0, :], axis=mybir.AxisListType.X
            ).then_inc(vd, 1)
            # tail chunk c1 (j1 cols 0:1568)
            j, s0, s1, tile, k = TAIL[1]
            vector.wait_ge(dt1, 16)
            vector.reduce_sum(
                stats[:, j, k : k + 1],
                tile[:, 0 : s1 - s0],
                axis=mybir.AxisListType.X,
            ).then_inc(vd, 1)

        @block.scalar
        def _(scalar: bass.BassEngine):
            # Q-scalar: b1, b3, b5 into fresh slots + tail chunks c1, c3;
            # all enqueued up front, no deps.
            for b, slot, sem in (
                (1, slots[1], ds1),
                (3, slots[3], ds3),
                (5, slots[5], ds5),
            ):
                scalar.dma_start(out=slot[:], in_=x[b]).then_inc(sem, 16)
            for i in (1, 3):
                j, s0, s1, tile, _k = TAIL[i]
                scalar.dma_start(
                    out=tile[:, 0 : s1 - s0], in_=x[7, :, j, s0:s1]
                ).then_inc(dt[i], 16)

            # ACT accum reduces: b1 pair, then b6-j1 (b6 streams 2nd on
            # Q-sync; DVE takes its j0), then b3, b5 pairs
            scalar.wait_ge(ds1, 16)
            for j in range(2):
                ins = scalar.activation(
                    slots[1][:, j, :],
                    slots[1][:, j, :],
                    mybir.ActivationFunctionType.Copy,
                    accum_out=stats[:, j, 1:2],
                )
                if j == 1:
                    ins.then_inc(ad, 1)
            scalar.wait_ge(ds6, 16)
            scalar.activation(
                slots[6][:, 1, :],
                slots[6][:, 1, :],
                mybir.ActivationFunctionType.Copy,
                accum_out=stats[:, 1, 6:7],
            ).then_inc(ad, 1)
            # tail c0 second half (j0 cols 1568: -> j0 col 8), hidden
            # mid-stream
            scalar.wait_ge(dt0, 16)
            scalar.activation(
                t0[:, 1568:HW],
                t0[:, 1568:HW],
                mybir.ActivationFunctionType.Copy,
                accum_out=stats[:, 0, 8:9],
            ).then_inc(ad, 1)
            for b, slot, sem in ((3, slots[3], ds3), (5, slots[5], ds5)):
                scalar.wait_ge(sem, 16)
                for j in range(2):
                    ins = scalar.activation(
                        slot[:, j, :],
                        slot[:, j, :],
                        mybir.ActivationFunctionType.Copy,
                        accum_out=stats[:, j, b : b + 1],
                    )
                    if j == 1:
                        ins.then_inc(ad, 1)
            # b4 j1 (DVE does j0 in parallel; b4 lands near stream end)
            scalar.wait_ge(ds4, 16)
            scalar.activation(
                slots[4][:, 1, :],
                slots[4][:, 1, :],
                mybir.ActivationFunctionType.Copy,
                accum_out=stats[:, 1, 4:5],
            ).then_inc(ad, 1)
            # tail chunks c2, c3
            for i in (2, 3):
                j, s0, s1, tile, k = TAIL[i]
                scalar.wait_ge(dt[i], 16)
                scalar.activation(
                    tile[:, 0 : s1 - s0],
                    tile[:, 0 : s1 - s0],
                    mybir.ActivationFunctionType.Copy,
                    accum_out=stats[:, j, k : k + 1],
                ).then_inc(ad, 1)

    _CACHE["nc"] = nc
    return nc


def kernel(layer_output, delay_keys, delay_values, in_channels, out_channels):
    global LAST_RESULTS
    _ensure_axon_hooks_shim()
    from concourse.bass_utils import run_bass_kernel_spmd

    x = np.ascontiguousarray(np.asarray(layer_output, dtype=np.float32))
    assert x.shape == (B_FULL, C, H, W), x.shape
    # shard over batch; view channels as (partition, pair): c = 2*p + j
    xr = x.reshape(N_CORES, B_LOCAL, 128, 2, HW)
    in_maps = [{"x": xr[k]} for k in range(N_CORES)]

    nc = _build()
    kwargs = {}
    if TRACE:
        kwargs.update(trace=True, tmpdir=TRACE_TMPDIR)
    res = run_bass_kernel_spmd(nc, in_maps, core_ids=list(range(N_CORES)), **kwargs)
    LAST_RESULTS = res

    # tiny [C] all-reduce of the per-core partial sums
    parts = np.stack(
        [res.results[k]["out"] for k in range(N_CORES)]
    )  # [8, 128, 2, 10]; j=0 valid cols 0..8, j=1 valid cols 0..9
    s0 = parts[:, :, 0, 0:9].sum(axis=(0, 2), dtype=np.float32)
    s1 = parts[:, :, 1, 0:10].sum(axis=(0, 2), dtype=np.float32)
    sums = np.stack([s0, s1], axis=1).reshape(C)  # c = 2p+j
    means = sums / np.float32(B_FULL * HW)
    means = np.round(means * np.float32(1e6)) / np.float32(1e6)

    keys = np.asarray(delay_keys, dtype=np.float32)
    values = np.asarray(delay_values, dtype=np.float32)
    K = keys.shape[0]
    idx = np.searchsorted(keys, means)
    lo = np.clip(idx - 1, 0, K - 1)
    hi = np.clip(idx, 0, K - 1)
    pick_hi = np.abs(keys[hi] - means) < np.abs(keys[lo] - means)
    nearest = np.where(pick_hi, hi, lo)
    merged = np.float32(values[nearest].max())

    scale = np.float32(
        (int(np.asarray(in_channels)) * int(np.asarray(out_channels))) / SCALE_DENOM
    )
    return np.full((H, W), merged, dtype=np.float32) * scale
